# revision 38
# baseline (speedup 1.0000x reference)
"""Trainium2 Bass kernel for nn_BiomechanicsLoss (masked quadratic-form loss).

Math (per point): et = [u0, v1, w2, .5(u1+v0), .5(u2+w0), .5(w1+v2)],
q = et^T C et with C = inv(compliance) cast to f32.  Loss =
sqrt(sum_masked(q^2)) / count_masked, mask = gt_sdf < 1e-8.

Because q = et^T C et == et^T sym(C) et and C is block-diagonal
(3x3 normal block + diagonal shear block), with F = diag(1,1,1,.5,.5,.5):
  q = w11*s1^2 + w22*s2^2 + w33*s3^2 + w12*s1*s2 + w13*s1*s3 + w23*s2*s3
      + d*(s4^2 + s5^2 + s6^2)
where s1..s3 = u0, v1, w2 ; s4 = u1+v0 ; s5 = u2+w0 ; s6 = w1+v2 and the
weights come from M = F*sym(C)*F (all positive for these constants).

Sharding: pure data-parallel over the N point dimension across 8 cores.
Each core reduces its shard to per-partition partials [128, 2T]
(T per-tile sum(mask*q^2) columns + T count columns); host does the final
tiny reduction, sqrt and divide.

Engine split per [128, F] tile (F=1024 free elems/partition):
  VectorE: 3 f32 adds (shear), 3 fused weighted products (tensor_tensor_reduce
           scale), 1 mask compare, 8 bf16 combine adds (2x mode), 1 bf16 q*m
  ScalarE: 6 weighted squares via activation(Square, scale=sqrt(w)),
           Square(q*m) with accum_out -> sum(mask*q^2), Identity(m) with
           accum_out -> count
  DMA:     contiguous [128, F, 3] tiles (12KB/partition) via HWDGE
"""

import numpy as np

N = 4_194_304
NCORES = 8
N_LOCAL = N // NCORES  # 524288
P = 128
J = N_LOCAL // P  # 4096 points per partition (partition-major layout)
# chunk widths; tapered head (compute starts sooner) and tail (short final
# serial chain)
CHUNKS = [512, 1024, 1024, 1024, 512]
NT = len(CHUNKS)
assert sum(CHUNKS) == J

THRESH = 1e-8


def _weights():
    vp, Ep = 0.4, 0.21
    Ci = np.zeros((6, 6), dtype=np.float64)
    Ci[0, 0] = 1 / Ep;  Ci[0, 1] = -vp / Ep; Ci[0, 2] = -vp / Ep
    Ci[1, 0] = -vp / Ep; Ci[1, 1] = 1 / Ep;  Ci[1, 2] = -vp / Ep
    Ci[2, 0] = -vp;      Ci[2, 1] = -vp;     Ci[2, 2] = 1 / Ep
    Ci[3, 3] = 2 * (1 + vp) / Ep
    Ci[4, 4] = Ci[3, 3]
    Ci[5, 5] = Ci[3, 3]
    # match reference: inverse computed in f64, cast to f32
    C = np.linalg.inv(Ci).astype(np.float32).astype(np.float64)
    Cs = 0.5 * (C + C.T)
    A = Cs[:3, :3]
    d = 0.25 * Cs[3, 3]
    return dict(
        w11=A[0, 0], w22=A[1, 1], w33=A[2, 2],
        w12=2 * A[0, 1], w13=2 * A[0, 2], w23=2 * A[1, 2],
        d=d,
    )


_NC = None


def _build_nc():
    import concourse.bacc as bacc
    import concourse.mybir as mybir
    import concourse.tile as tile

    W = _weights()
    r11 = float(np.sqrt(W["w11"]))
    r22 = float(np.sqrt(W["w22"]))
    r33 = float(np.sqrt(W["w33"]))
    rd = float(np.sqrt(W["d"]))
    # factor cross weights: w12 = a1*a2, w13 = a1*a3, w23 = a2*a3 so the
    # products use pre-scaled bf16 copies (all bf16 -> DVE 2x mode)
    a1s = float(np.sqrt(W["w12"] * W["w13"] / W["w23"]))
    a2s = float(W["w12"] / a1s)
    a3s = float(W["w13"] / a1s)

    f32 = mybir.dt.float32
    bf16 = mybir.dt.bfloat16
    Sq = mybir.ActivationFunctionType.Square
    Ident = mybir.ActivationFunctionType.Identity
    ALU = mybir.AluOpType

    nc = bacc.Bacc()
    # host packs each core's shard chunk-major: for each chunk t, partition p:
    # [u (3F interleaved) | v (3F) | w (3F) | sd (F)] -> one contiguous DMA
    # per chunk (4MB-class, ~97% DMA efficiency)
    packed = nc.dram_tensor("packed", [P, 10 * J], f32, kind="ExternalInput")
    out = nc.dram_tensor("out", [P, 2 * NT], f32, kind="ExternalOutput")

    with tile.TileContext(nc) as tc:
        with (
            tc.tile_pool(name="io", bufs=3) as io,
            tc.tile_pool(name="mid", bufs=2) as mid,
            tc.tile_pool(name="stats", bufs=1) as stats_pool,
        ):
            stats = stats_pool.tile([P, 2 * NT], f32)

            c0 = 0
            for t, F in enumerate(CHUNKS):
                buf = io.tile([P, 10 * F], f32, tag="buf")
                nc.sync.dma_start(out=buf[:], in_=packed[:, c0:c0 + 10 * F])
                c0 += 10 * F

                # host-packed chunk layout (all contiguous [P, F] blocks):
                # [u0 v1 w2 | u1 v0 u2 w0 w1 v2 | sd]
                u0v1 = buf[:, 0 * F:2 * F]
                w2 = buf[:, 2 * F:3 * F]
                u1, v0 = buf[:, 3 * F:4 * F], buf[:, 4 * F:5 * F]
                u2, w0 = buf[:, 5 * F:6 * F], buf[:, 6 * F:7 * F]
                w1, v2 = buf[:, 7 * F:8 * F], buf[:, 8 * F:9 * F]
                sd = buf[:, 9 * F:10 * F]

                # shear strain components into one [P,3F] tile
                # (f32 contiguous in, bf16 out)
                s456 = mid.tile([P, 3 * F], bf16, tag="s456")
                nc.vector.tensor_add(s456[:, 0:F], u1, v0)
                nc.vector.tensor_add(s456[:, F:2 * F], u2, w0)
                nc.vector.tensor_add(s456[:, 2 * F:3 * F], w1, v2)

                # pre-scaled bf16 copies on ScalarE (alpha1 == alpha2, so
                # u0 and v1 share one 2F-wide copy)
                p12 = mid.tile([P, 2 * F], bf16, tag="p12")
                p3 = mid.tile([P, F], bf16, tag="p3")
                nc.scalar.mul(p12, u0v1, a1s)
                nc.scalar.mul(p3, w2, a3s)

                # mask (f32 single-src 2x); fused row-sum accum = count
                m = mid.tile([P, F], bf16, tag="m")
                nc.vector.tensor_scalar(
                    out=m, in0=sd, scalar1=THRESH, scalar2=None, op0=ALU.is_lt,
                    op1=ALU.add, accum_out=stats[:, NT + t:NT + t + 1])

                # term tiles: X = [z4 z5 z6 | z3], Y1 = [z1 z2], Y2 = [ca cb]
                X = mid.tile([P, 4 * F], bf16, tag="X")
                Y1 = mid.tile([P, 2 * F], bf16, tag="Y1")
                Y2 = mid.tile([P, 2 * F], bf16, tag="Y2")

                # weighted squares on ScalarE (wide ops; r11 == r22)
                nc.scalar.activation(X[:, 0:3 * F], s456, Sq, scale=rd)
                nc.scalar.activation(X[:, 3 * F:4 * F], p3, Sq, scale=r33 / a3s)
                nc.scalar.activation(Y1, p12, Sq, scale=r11 / a1s)

                # cross products, factored: p1p2 + p1p3 + p2p3 =
                # p1*(p2+p3) + p2*p3  (all bf16, DVE 2x)
                tp = mid.tile([P, F], bf16, tag="tp")
                nc.vector.tensor_add(tp, p12[:, F:2 * F], p3)
                nc.vector.tensor_mul(Y2[:, 0:F], p12[:, 0:F], tp)
                nc.vector.tensor_mul(Y2[:, F:2 * F], p12[:, F:2 * F], p3)

                # combine 8 terms with a 3-level wide fold (work 7F, 4 ops)
                nc.vector.tensor_add(Y1, Y1, Y2)                    # 2F
                nc.vector.tensor_add(X[:, 0:2 * F], X[:, 0:2 * F],
                                     X[:, 2 * F:4 * F])             # 2F
                nc.vector.tensor_add(Y1, Y1, X[:, 0:2 * F])         # 2F
                q = p3  # reuse consumed tile for q
                nc.vector.tensor_add(q, Y1[:, 0:F], Y1[:, F:2 * F])  # F

                # qm = q * mask (bf16 2x), then ssq via fused square+row-sum
                nc.vector.tensor_mul(m, q, m)
                junk1 = mid.tile([P, F], bf16, tag="junk1")
                nc.scalar.activation(
                    junk1, m, Sq, accum_out=stats[:, t:t + 1])

            nc.sync.dma_start(out=out[:, :], in_=stats[:])

    nc.compile()
    return nc


def _get_nc():
    global _NC
    if _NC is None:
        _NC = _build_nc()
    return _NC


def _run(in_maps, trace=False, **kwargs):
    from concourse.bass_utils import run_bass_kernel_spmd

    nc = _get_nc()
    return run_bass_kernel_spmd(
        nc, in_maps, core_ids=list(range(NCORES)), trace=trace, **kwargs)


def _make_in_maps(grad_u, grad_v, grad_w, gt_sdf):
    grad_u = np.asarray(grad_u, dtype=np.float32)
    grad_v = np.asarray(grad_v, dtype=np.float32)
    grad_w = np.asarray(grad_w, dtype=np.float32)
    gt_sdf = np.asarray(gt_sdf, dtype=np.float32)
    in_maps = []
    for c in range(NCORES):
        sl = slice(c * N_LOCAL, (c + 1) * N_LOCAL)
        gu = grad_u[sl].reshape(P, J, 3)
        gv = grad_v[sl].reshape(P, J, 3)
        gw = grad_w[sl].reshape(P, J, 3)
        sd = gt_sdf[sl].reshape(P, J)
        parts = []
        off = 0
        for F in CHUNKS:
            s = slice(off, off + F)
            parts += [gu[:, s, 0], gv[:, s, 1], gw[:, s, 2],
                      gu[:, s, 1], gv[:, s, 0],
                      gu[:, s, 2], gw[:, s, 0],
                      gw[:, s, 1], gv[:, s, 2],
                      sd[:, s]]
            off += F
        packed = np.ascontiguousarray(np.concatenate(parts, axis=1))
        in_maps.append({"packed": packed})
    return in_maps


def _finalize(results):
    ssq = 0.0
    cnt = 0.0
    for res in results:
        st = np.asarray(res["out"], dtype=np.float64)
        ssq += st[:, :NT].sum()
        cnt += st[:, NT:].sum()
    Wv = np.sqrt(ssq)
    return np.float32(Wv / cnt)


def kernel(grad_u, grad_v, grad_w, gt_sdf):
    in_maps = _make_in_maps(grad_u, grad_v, grad_w, gt_sdf)
    res = _run(in_maps, trace=False)
    return _finalize(res.results)
